# revision 37
# baseline (speedup 1.0000x reference)
"""Trainium2 Bass kernel for nn_Attention (GQA with group-summed query heads).

Algorithm notes (validated against reference in numpy):
- The reference einsum 'bghnd,bhsd->bhns' SUMS over the query-group axis, so the
  16 query heads collapse into 4 effective heads: wq columns can be pre-summed
  per kv-head (RoPE is linear per-position, both /sqrt(64) scalings folded in).
- This makes the problem plain 4-head attention: 2 batches x 4 kv-heads = 8
  independent (b,h) attention instances -> one per NeuronCore.
- Head dims are pair-permuted [t1(even), t2(odd)] so RoPE becomes wide
  elementwise multiply-adds: P1 = W1.T@xT (q/k stacked on 32-row blocks),
  P2 = signed pair-swap of P1 done by ONE permutation-matrix matmul, then
  rot = P1*[c;c;c;c] + P2*[s;s;s;s].
- Scores are computed TRANSPOSED (scoresT[key, query]) so exp(scoresT) is
  directly usable as the AV matmul's stationary-side rhs with V as lhsT; an
  all-ones column appended to V yields the softmax denominators for free.
- No max-subtraction needed: scores = q_eff . k / 64 with |scores| <~ 1.5.
- Softmax denominators come free via an all-ones column appended to V; the
  normalization happens BEFORE the AllToAll (reciprocal of the sums row +
  ones-outer-product matmul broadcast), so the collective carries bf16 [64,512]
  blocks only.
- Final: 8-core AllToAll redistributes per-head outputs into 512-row output
  slices; each core applies the row-parallel wo matmul with head-pairs stacked
  to K=128.

Performance notes:
- ALL bulk DMAs go through the two HWDGE rings (nc.sync = SP, nc.scalar = ACT).
  The original used nc.gpsimd (SWDGE): Q7 software descriptor generation for
  the strided x load (1024 x 1KB runs per MB) cost ~1.5 ms alone - ~6x the
  whole kernel's compute. Weights are pre-cast to bf16 on host so no cast-DMA
  (gpsimd-only) is ever needed.
- iters=K builds the whole body K times inside one NEFF (same SBUF slots,
  dependency-serialized). Used by test.py to measure pure on-device time as
  the slope over K, cancelling the ~80 ms axon-tunnel dispatch round-trip.
"""

import sys
import os

sys.path.insert(0, "/opt/trn_rl_repo")

import numpy as np
import ml_dtypes

B, S, D = 2, 4096, 1024
QH, KVH, HD = 16, 4, 64
KV_DIM = KVH * HD  # 256
NCORES = 8
SB = 512           # s-block / q-block width
NSB = S // SB      # 8
GRP = 2            # key-chunks (128 keys each) per exp group -> [128, 1024] psum

_CACHE = {}


def _build_nc(stop_after="D", collective=True, iters=1, pulls=8, ldmode=1):
    import concourse.bacc as bacc
    import concourse.tile as tile
    from concourse import mybir

    f32 = mybir.dt.float32
    bf = mybir.dt.bfloat16
    EXP = mybir.ActivationFunctionType.Exp

    nc = bacc.Bacc("TRN2", target_bir_lowering=False, debug=False,
                   num_devices=NCORES)

    xT_d = nc.dram_tensor("xT", [D, S], bf, kind="ExternalInput")
    w1_d = nc.dram_tensor("w1", [D, 128], bf, kind="ExternalInput")
    wv_d = nc.dram_tensor("wv", [D, HD], bf, kind="ExternalInput")
    ab_d = nc.dram_tensor("ab", [2, 128, S], bf, kind="ExternalInput")
    mk_d = nc.dram_tensor("masks", [128, 128], bf, kind="ExternalInput")
    wo_d = nc.dram_tensor("wo", [KV_DIM, D], bf, kind="ExternalInput")
    out_d = nc.dram_tensor("out", [B, SB, D], bf, kind="ExternalOutput")

    idn_d = nc.inline_tensor(np.eye(HD, dtype=np.float32), "idn")
    Mperm = np.zeros((128, 128), np.float32)
    for r in range(0, 32):
        Mperm[r, r + 32] = -1.0
        Mperm[r + 32, r] = 1.0
        Mperm[r + 64, r + 96] = -1.0
        Mperm[r + 96, r + 64] = 1.0
    permT_d = nc.inline_tensor(Mperm.T.astype(ml_dtypes.bfloat16), "permT")
    # partition-halves swap: rotB[p] = rot[(p+64)%128] via one PE matmul
    Mswap = np.zeros((128, 128), np.float32)
    for p in range(128):
        Mswap[(p + 64) % 128, p] = 1.0
    swapT_d = nc.inline_tensor(Mswap.astype(ml_dtypes.bfloat16), "swapT")

    lvl = ["L", "P", "A", "C", "D"].index(stop_after)

    with tile.TileContext(nc) as tc:
        with (
            tc.tile_pool(name="persist", bufs=1) as pp,
            tc.tile_pool(name="work", bufs=4) as wp,
            tc.tile_pool(name="expp", bufs=4) as ep,
            tc.tile_pool(name="ps_sc", bufs=2, space="PSUM") as ps_sc,
            tc.tile_pool(name="ps_p", bufs=1, space="PSUM") as ps_p,
            tc.tile_pool(name="ps_aux", bufs=1, space="PSUM") as ps_aux,
            tc.tile_pool(name="dram", bufs=1, space="DRAM") as dp,
        ):
            # ---- persistent SBUF tensors ----
            xT = pp.tile([128, 8 * S], bf, tag="xT")          # 64KB/part
            w1 = pp.tile([128, 8 * 128], bf, tag="w1")
            wv = pp.tile([128, 8 * HD], bf, tag="wv")
            At = pp.tile([128, S], bf, tag="At")
            Bt = pp.tile([128, S], bf, tag="Bt")
            mk = pp.tile([128, 128], bf, tag="mk")  # one causal triangle
            wo = pp.tile([128, 2 * D], bf, tag="wo")          # head-pair c rows 128c
            rot = pp.tile([128, S], bf, tag="rot")            # rows 0:64 q~, 64:128 k~
            rotB = pp.tile([128, S], bf, tag="rotB")          # swapped: k~ low, q~ high
            VS = HD + 1
            vaug = pp.tile([128, 32 * VS], bf, tag="vaug")
            outTs = []
            for q in range(NSB):
                oT = pp.tile([HD, SB], bf, tag=f"outT{q}")
                outTs.append(oT)
            idn = pp.tile([HD, HD], f32, tag="idn")
            permT = pp.tile([128, 128], bf, tag="permT")
            swapT = pp.tile([128, 128], bf, tag="swapT")
            ones65 = pp.tile([HD + 1, HD], f32, tag="ones65")
            wdbg = None
            if lvl < 4:
                # debug levels: tiny constant output write (timing-only levels;
                # content is irrelevant, keep it off the DMA-ring hot path)
                wdbg = pp.tile([128, 16], bf, tag="wdbg")

            def emit_body():
                # ---- input loads (all HWDGE: sync=SP ring for x, scalar=ACT
                # ring for weights/tables; all pre-cast to bf16 on host) ----
                # weights first (small, needed by the first projection)
                w1_3 = w1[:, :].rearrange("p (c m) -> p c m", c=8)
                w1d3 = w1_d[:, :].rearrange("(c p) m -> p c m", p=128)
                nc.scalar.dma_start(w1_3[:, :, :], w1d3[:, :, :])
                # first x s-block on the sync ring, split in two so P(0)'s
                # first d-chunk matmuls can begin at the half-load mark
                xT3 = xT[:, :].rearrange("p (c s) -> p c s", c=8)
                xTd3 = xT_d[:, :].rearrange("(c p) s -> p c s", p=128)
                nc.sync.dma_start(xT3[:, 0:4, 0: SB], xTd3[:, 0:4, 0: SB])
                nc.sync.dma_start(xT3[:, 4:8, 0: SB], xTd3[:, 4:8, 0: SB])
                wv_3 = wv[:, :].rearrange("p (c m) -> p c m", c=8)
                wvd3 = wv_d[:, :].rearrange("(c p) m -> p c m", p=128)
                wo_3 = wo[:, :].rearrange("p (c n) -> p c n", c=2)
                wod3 = wo_d[:, :].rearrange("(c p) n -> p c n", p=128)
                if ldmode == 0:
                    # whole-table loads on the ACT ring
                    nc.scalar.dma_start(wv_3[:, :, :], wvd3[:, :, :])
                    nc.scalar.dma_start(At[:, :], ab_d[0, :, :])
                    nc.scalar.dma_start(Bt[:, :], ab_d[1, :, :])
                    nc.scalar.dma_start(mk[:, :], mk_d[:, :])
                else:
                    # latency-ordered ACT ring: only block-0 table slices up
                    # front; later At/Bt blocks ride inside the P(j) generators
                    nc.scalar.dma_start(At[:, 0:SB], ab_d[0, :, 0:SB])
                    nc.scalar.dma_start(Bt[:, 0:SB], ab_d[1, :, 0:SB])
                    nc.scalar.dma_start(wv_3[:, :, :], wvd3[:, :, :])
                    nc.scalar.dma_start(mk[:, :], mk_d[:, :])
                nc.scalar.dma_start(idn[:, :], idn_d[:, :])
                nc.scalar.dma_start(permT[:, :], permT_d[:, :])
                nc.scalar.dma_start(swapT[:, :], swapT_d[:, :])
                nc.vector.memset(ones65[:, :], 1.0)
                for c in range(32):
                    nc.vector.memset(vaug[:, VS * c + HD: VS * c + HD + 1], 1.0)
                # s-sliced x loads: one DMA brings ALL 8 D-chunks for one
                # s-block, so block-j projection starts after load j
                for j in range(1, NSB):
                    nc.sync.dma_start(xT3[:, :, SB * j: SB * (j + 1)],
                                      xTd3[:, :, SB * j: SB * (j + 1)])

                if lvl < 1:
                    nc.vector.memset(wdbg[:, :], 1.0)
                    nc.sync.dma_start(out_d[0, 0:128, 0:16], wdbg[:, :])
                if lvl >= 3:
                    bin_ = dp.tile([NCORES, HD, 512], bf, tag="bin")
                    bout = dp.tile([NCORES, HD, 512], bf, tag="bout")

                # ---- stages P+A interleaved per s-block ----
                # P(j): projections + RoPE + V-transpose for s-block j.
                # A(qb=j): full attention row-band for q-block j (needs rot/v of
                # s-blocks 0..j only, all available after P(j)).
                # P(j+1) is emitted as a GENERATOR whose steps are pulled a few
                # at a time inside A(j)'s group loop: A is ACT(exp)-bound with
                # PE slack, so P's matmuls hide under the exp stream instead of
                # serializing between attention bands.
                def emit_P_steps(j):
                    sc = ps_p.tile([128, 3 * SB], f32, tag="pp")
                    if ldmode != 0 and j >= 1:
                        nc.scalar.dma_start(At[:, SB * j: SB * (j + 1)],
                                            ab_d[0, :, SB * j: SB * (j + 1)])
                        nc.scalar.dma_start(Bt[:, SB * j: SB * (j + 1)],
                                            ab_d[1, :, SB * j: SB * (j + 1)])

                    def xs(d8, _j=j):
                        return xT[:, S * d8 + SB * _j: S * d8 + SB * (_j + 1)]

                    for d8 in range(8):
                        nc.tensor.matmul(sc[:, 0:512],
                                         w1[:, 128 * d8: 128 * (d8 + 1)],
                                         xs(d8), start=(d8 == 0), stop=(d8 == 7))
                        yield
                    for d8 in range(8):
                        nc.tensor.matmul(sc[0:HD, 1024:1536],
                                         wv[:, HD * d8: HD * (d8 + 1)],
                                         xs(d8), start=(d8 == 0), stop=(d8 == 7))
                        yield
                    # P2 = signed pair-swap of P1 -> ONE permutation matmul
                    p1s = wp.tile([128, SB], bf, tag="p1s")
                    nc.vector.tensor_copy(p1s[:, :], sc[:, 0:512])
                    nc.tensor.matmul(sc[:, 512:1024], permT[:, :], p1s[:, :],
                                     start=True, stop=True)
                    yield
                    u = wp.tile([128, SB], bf, tag="u")
                    w_ = wp.tile([128, SB], f32, tag="w_")
                    nc.vector.tensor_mul(u[:, :], p1s[:, :],
                                         At[:, SB * j: SB * (j + 1)])
                    nc.vector.tensor_mul(w_[:, :], sc[:, 512:1024],
                                         Bt[:, SB * j: SB * (j + 1)])
                    nc.vector.tensor_add(rot[:, SB * j: SB * (j + 1)],
                                         u[:, :], w_[:, :])
                    yield
                    # partition-swapped copy rotB (k~ low / q~ high) via one PE
                    # matmul + one DVE copy: no DMA on any ring, so the ACT
                    # queue stays pure exp during attention (head-of-line!)
                    nc.tensor.matmul(sc[:, 512:1024], swapT[:, :],
                                     rot[:, SB * j: SB * (j + 1)],
                                     start=True, stop=True)
                    yield
                    nc.vector.tensor_copy(rotB[:, SB * j: SB * (j + 1)],
                                          sc[:, 512:1024])
                    yield
                    vts = wp.tile([HD, SB], f32, tag="vts")
                    nc.vector.tensor_copy(vts[:, :], sc[0:HD, 1024:1536])
                    yield
                    # transposes reuse this j's pp bank 0 (P1 already consumed)
                    for t in range(4):
                        nc.tensor.transpose(sc[:, HD * t: HD * (t + 1)],
                                            vts[:, 128 * t: 128 * (t + 1)],
                                            idn[:, :])
                        yield
                    for t in range(4):
                        cch = 4 * j + t
                        nc.vector.tensor_copy(vaug[:, VS * cch: VS * cch + HD],
                                              sc[:, HD * t: HD * (t + 1)])

                def drain(gen):
                    if gen is not None:
                        for _ in gen:
                            pass

                if lvl >= 1:
                    drain(emit_P_steps(0))
                for qb in range(NSB if lvl >= 2 else 0):
                    if qb >= 1 and pulls == 0:
                        drain(emit_P_steps(qb))  # serial P(j);A(j) ordering
                    pgen = (emit_P_steps(qb + 1)
                            if (qb + 1 < NSB and pulls > 0) else None)
                    # ---- A(qb) ----
                    po = ps_aux.tile([HD + 1, 512], f32, tag="po")
                    nk = 4 * (qb + 1)
                    for g0 in range(0, nk, GRP):
                        cnt = min(GRP, nk - g0)
                        sc = ps_sc.tile([128, GRP * SB], f32, tag="sc")
                        for r in range(cnt):
                            kb = g0 + r
                            # diagonal chunk di>=1: queries < 128*di are fully
                            # masked -> restrict scores/AV to cols >= 128*di
                            di = kb - (nk - 4)
                            lo = 128 * di if di >= 1 else 0
                            dst = sc[:, 512 * r + lo: 512 * (r + 1)]
                            if r % 2 == 0:
                                # row-tile T0: k~ (rotB low) x q~ (rot low)
                                nc.tensor.matmul(
                                    dst, rotB[0:HD, 128 * kb: 128 * (kb + 1)],
                                    rot[0:HD, SB * qb + lo: SB * (qb + 1)],
                                    start=True, stop=True)
                            else:
                                # row-tile T8: k~ (rot high) x q~ (rotB high)
                                nc.tensor.matmul(
                                    dst, rot[64:128, 128 * kb: 128 * (kb + 1)],
                                    rotB[64:128, SB * qb + lo: SB * (qb + 1)],
                                    start=True, stop=True)
                        pe = ep.tile([128, GRP * SB], bf, tag="pe")
                        nc.scalar.activation(pe[:, 0: 512 * cnt],
                                             sc[:, 0: 512 * cnt], EXP)
                        # pull a few P(qb+1) steps: their PE matmuls land
                        # between this group's scores and AV in the in-order
                        # PE queue, filling the wait for exp(group) to finish
                        if pgen is not None:
                            for _ in range(pulls):
                                if next(pgen, "end") == "end":
                                    pgen = None
                                    break
                        for r in range(cnt):
                            di = (g0 + r) - (nk - 4)
                            if di >= 0:
                                # only the 128-wide triangle strip needs the
                                # mask (cols < 128di skipped by scores/AV,
                                # cols >= 128(di+1) are all-valid)
                                o = 512 * r + 128 * di
                                nc.vector.tensor_mul(pe[:, o: o + 128],
                                                     pe[:, o: o + 128],
                                                     mk[:, :])
                        for r in range(cnt):
                            kb = g0 + r
                            di = kb - (nk - 4)
                            lo = 128 * di if di >= 1 else 0
                            nc.tensor.matmul(po[:, lo: 512],
                                             vaug[:, VS * kb: VS * kb + HD + 1],
                                             pe[:, 512 * r + lo: 512 * (r + 1)],
                                             start=(kb == 0), stop=(kb == nk - 1))
                    drain(pgen)
                    # normalize locally BEFORE the a2a: recip of the sums row
                    # (partition 64) into SBUF, matmul-broadcast to 64 parts
                    rcs = wp.tile([HD + 1, 512], f32, tag="rcs")
                    nc.vector.reciprocal(rcs[HD:HD + 1, :], po[HD:HD + 1, :])
                    bc = ps_sc.tile([HD, 512], f32, tag="sc")
                    nc.tensor.matmul(bc[:, :], ones65[HD:HD + 1, :],
                                     rcs[HD:HD + 1, :], start=True, stop=True)
                    # walrus: a DVE op may read at most ONE PSUM operand
                    nc.any.tensor_copy(outTs[qb][:, :], po[0:HD, :])
                    nc.vector.tensor_mul(outTs[qb][:, :], outTs[qb][:, :],
                                         bc[:, :])
                    if lvl >= 3:
                        # eager bounce-out: slice qb of outT = a2a block qb
                        nc.sync.dma_start(bin_[qb, :, :], outTs[qb][:, :])
                if lvl == 1:
                    for j in range(1, NSB):
                        drain(emit_P_steps(j))

                # ---- stage C: AllToAll (512-col output slices, all 8 cores) --
                if lvl in (1, 2):
                    nc.vector.memset(wdbg[:, :], 1.0)
                    nc.sync.dma_start(out_d[0, 0:128, 0:16], wdbg[:, :])
                if lvl >= 3:
                    if collective:
                        from concourse import mybir as _mb
                        nc.gpsimd.collective_compute(
                            "AllToAll", _mb.AluOpType.bypass,
                            replica_groups=[list(range(NCORES))],
                            ins=[bin_.opt()], outs=[bout.opt()],
                        )
                    else:
                        # single-core timeline-sim stand-in: local DRAM move
                        nc.sync.dma_start(bout[:, :, :], bin_[:, :, :])
                    if lvl == 3:
                        nc.vector.memset(wdbg[:, :], 1.0)
                        nc.sync.dma_start(out_d[0, 0:128, 0:16], wdbg[:, :])

                # ---- stage D: wo matmul + output (already normalized) ----
                if lvl >= 4:
                    # wo load deferred to here: first needed by the D matmuls,
                    # keeps it off the early HBM window and the ACT ring short
                    nc.scalar.dma_start(wo_3[:, :, :], wod3[:, :, :])
                    # head pairs stacked on partition halves -> K=128 wo matmul
                    g2 = pp.tile([128, 4 * 512], bf, tag="g2")
                    g2lo = g2[0:HD, :].rearrange("p (m s) -> p m s", m=4)
                    g2hi = g2[HD:128, :].rearrange("p (m s) -> p m s", m=4)
                    bt3 = bout[:, :, :].rearrange("(m e) p s -> e p m s", e=2)
                    nc.sync.dma_start(g2lo[:, :, :], bt3[0, :, :, :])
                    nc.scalar.dma_start(g2hi[:, :, :], bt3[1, :, :, :])
                    for b in range(B):
                        for t in range(4):
                            # bf16 staging halves both the output DMA and the
                            # host fetch; rel-err cost ~4e-3, well inside gate
                            ys = wp.tile([128, 1024], bf, tag="ys")
                            for nh in range(2):
                                # stage-A sc slots are free here; reuse for 2x
                                # double-buffered wo-psum
                                yp = ps_sc.tile([128, 512], f32, tag="sc")
                                for pr in range(2):
                                    m = 2 * b + pr
                                    nc.tensor.matmul(
                                        yp[:, :],
                                        g2[:, 512 * m + 128 * t:
                                           512 * m + 128 * (t + 1)],
                                        wo[:, D * pr + 512 * nh:
                                           D * pr + 512 * (nh + 1)],
                                        start=(pr == 0), stop=(pr == 1))
                                nc.any.tensor_copy(ys[:, 512 * nh:
                                                      512 * (nh + 1)], yp[:, :])
                            eng = (nc.sync, nc.scalar)[(4 * b + t) % 2]
                            eng.dma_start(out_d[b, 128 * t: 128 * (t + 1), :],
                                          ys[:, :])

            for _it in range(iters):
                emit_body()

    nc.compile()
    return nc


def _get_nc():
    if "nc" not in _CACHE:
        _CACHE["nc"] = _build_nc()
    return _CACHE["nc"]


def _prep_in_maps(x, wq, wk, wv, wo, freq_cos, freq_sin):
    x = np.asarray(x, np.float32)
    wq = np.asarray(wq, np.float32)
    wk = np.asarray(wk, np.float32)
    wv = np.asarray(wv, np.float32)
    wo = np.asarray(wo, np.float32)
    cos = np.asarray(freq_cos, np.float32)
    sin = np.asarray(freq_sin, np.float32)

    # group-sum wq per kv head (einsum sums over group axis); fold both /8 scales
    wqr = wq.reshape(D, QH, HD)
    wq_eff = np.stack([wqr[:, h::KVH].sum(axis=1) for h in range(KVH)], axis=1) / 64.0
    wkr = wk.reshape(D, KVH, HD)
    W1 = np.empty((KVH, D, 128), np.float32)
    for h in range(KVH):
        q1, q2 = wq_eff[:, h, 0::2], wq_eff[:, h, 1::2]
        k1, k2 = wkr[:, h, 0::2], wkr[:, h, 1::2]
        W1[h] = np.concatenate([q1, q2, k1, k2], axis=1)
    Wv = np.ascontiguousarray(wv.reshape(D, KVH, HD).transpose(1, 0, 2))

    A = np.tile(cos.T, (4, 1)).astype(ml_dtypes.bfloat16)   # [128, S]
    Bm = np.tile(sin.T, (4, 1)).astype(ml_dtypes.bfloat16)
    ab = np.ascontiguousarray(np.stack([A, Bm]))

    qi = np.arange(128)[None, :]
    ki = np.arange(128)[:, None]
    masks = np.ascontiguousarray((qi >= ki).astype(ml_dtypes.bfloat16))

    xTb = [np.ascontiguousarray(x[b].T).astype(ml_dtypes.bfloat16) for b in range(B)]
    wo_bf = np.ascontiguousarray(wo.astype(ml_dtypes.bfloat16))

    in_maps = []
    for c in range(NCORES):
        b, h = c // KVH, c % KVH
        in_maps.append({
            "xT": xTb[b],
            "w1": np.ascontiguousarray(W1[h].astype(ml_dtypes.bfloat16)),
            "wv": np.ascontiguousarray(Wv[h].astype(ml_dtypes.bfloat16)),
            "ab": ab,
            "masks": masks,
            "wo": wo_bf,
        })
    return in_maps


def _assemble(results):
    full = np.empty((B, S, D), np.float32)
    for c in range(NCORES):
        y = results[c]["out"]  # [B, 512, D] bf16
        for b in range(B):
            full[b, SB * c: SB * (c + 1), :] = y[b].astype(np.float32)
    return full


def _ensure_axon_hooks_stub():
    # slim axon builds lack antenv.axon_hooks; degrade trace=True gracefully
    try:
        import antenv.axon_hooks  # noqa: F401
    except Exception:
        import types
        m = types.ModuleType("antenv.axon_hooks")
        m.get_axon_ntff_profile_hook = lambda: None
        sys.modules["antenv.axon_hooks"] = m


def run(in_maps, trace=False):
    from concourse.bass_utils import run_bass_kernel_spmd
    _ensure_axon_hooks_stub()
    nc = _get_nc()
    res = run_bass_kernel_spmd(nc, in_maps, core_ids=list(range(NCORES)),
                               trace=trace)
    return res


def kernel(**inputs):
    in_maps = _prep_in_maps(**inputs)
    res = run(in_maps, trace=False)
    return _assemble(res.results)


if __name__ == "__main__":
    # smoke: build only
    _get_nc()
    print("built ok")


# revision 42
# speedup vs baseline: 1435.3033x; 1435.3033x over previous
"""Trainium2 Bass kernel for nn_Attention (GQA with group-summed query heads).

Algorithm notes (validated against reference in numpy):
- The reference einsum 'bghnd,bhsd->bhns' SUMS over the query-group axis, so the
  16 query heads collapse into 4 effective heads: wq columns can be pre-summed
  per kv-head (RoPE is linear per-position, both /sqrt(64) scalings folded in).
- This makes the problem plain 4-head attention: 2 batches x 4 kv-heads = 8
  independent (b,h) attention instances -> one per NeuronCore.
- Head dims are pair-permuted [t1(even), t2(odd)] so RoPE becomes wide
  elementwise multiply-adds: P1 = W1.T@xT (q/k stacked on 32-row blocks),
  P2 = signed pair-swap of P1 done by ONE permutation-matrix matmul, then
  rot = P1*[c;c;c;c] + P2*[s;s;s;s].
- Scores are computed TRANSPOSED (scoresT[key, query]) so exp(scoresT) is
  directly usable as the AV matmul's stationary-side rhs with V as lhsT; an
  all-ones column appended to V yields the softmax denominators for free.
- No max-subtraction needed: scores = q_eff . k / 64 with |scores| <~ 1.5.
- Softmax denominators come free via an all-ones column appended to V; the
  normalization happens BEFORE the AllToAll (reciprocal of the sums row +
  ones-outer-product matmul broadcast), so the collective carries bf16 [64,512]
  blocks only.
- Final: 8-core AllToAll redistributes per-head outputs into 512-row output
  slices; each core applies the row-parallel wo matmul with head-pairs stacked
  to K=128.

Performance notes:
- ALL bulk DMAs go through the two HWDGE rings (nc.sync = SP, nc.scalar = ACT).
  The original used nc.gpsimd (SWDGE): Q7 software descriptor generation for
  the strided x load (1024 x 1KB runs per MB) cost ~1.5 ms alone - ~6x the
  whole kernel's compute. Weights are pre-cast to bf16 on host so no cast-DMA
  (gpsimd-only) is ever needed.
- iters=K builds the whole body K times inside one NEFF (same SBUF slots,
  dependency-serialized). Used by test.py to measure pure on-device time as
  the slope over K, cancelling the ~80 ms axon-tunnel dispatch round-trip.
"""

import sys
import os

sys.path.insert(0, "/opt/trn_rl_repo")

import numpy as np
import ml_dtypes

B, S, D = 2, 4096, 1024
QH, KVH, HD = 16, 4, 64
KV_DIM = KVH * HD  # 256
NCORES = 8
SB = 512           # s-block / q-block width
NSB = S // SB      # 8
GRP = 2            # key-chunks (128 keys each) per exp group -> [128, 1024] psum

_CACHE = {}


def _build_nc(stop_after="D", collective=True, iters=1, pulls=8, ldmode=1):
    import concourse.bacc as bacc
    import concourse.tile as tile
    from concourse import mybir

    f32 = mybir.dt.float32
    bf = mybir.dt.bfloat16
    EXP = mybir.ActivationFunctionType.Exp

    nc = bacc.Bacc("TRN2", target_bir_lowering=False, debug=False,
                   num_devices=NCORES)

    xT_d = nc.dram_tensor("xT", [D, S], bf, kind="ExternalInput")
    w1_d = nc.dram_tensor("w1", [D, 128], bf, kind="ExternalInput")
    wv_d = nc.dram_tensor("wv", [D, HD], bf, kind="ExternalInput")
    ab_d = nc.dram_tensor("ab", [2, 128, S], bf, kind="ExternalInput")
    mk_d = nc.dram_tensor("masks", [128, 128], bf, kind="ExternalInput")
    wo_d = nc.dram_tensor("wo", [KV_DIM, D], bf, kind="ExternalInput")
    out_d = nc.dram_tensor("out", [B, SB, D], bf, kind="ExternalOutput")

    idn_d = nc.inline_tensor(np.eye(HD, dtype=np.float32), "idn")
    Mperm = np.zeros((128, 128), np.float32)
    for r in range(0, 32):
        Mperm[r, r + 32] = -1.0
        Mperm[r + 32, r] = 1.0
        Mperm[r + 64, r + 96] = -1.0
        Mperm[r + 96, r + 64] = 1.0
    permT_d = nc.inline_tensor(Mperm.T.astype(ml_dtypes.bfloat16), "permT")
    # partition-halves swap: rotB[p] = rot[(p+64)%128] via one PE matmul
    Mswap = np.zeros((128, 128), np.float32)
    for p in range(128):
        Mswap[(p + 64) % 128, p] = 1.0
    swapT_d = nc.inline_tensor(Mswap.astype(ml_dtypes.bfloat16), "swapT")

    lvl = ["L", "P", "A", "C", "D"].index(stop_after)

    with tile.TileContext(nc) as tc:
        with (
            tc.tile_pool(name="persist", bufs=1) as pp,
            tc.tile_pool(name="work", bufs=4) as wp,
            tc.tile_pool(name="expp", bufs=4) as ep,
            tc.tile_pool(name="ps_sc", bufs=2, space="PSUM") as ps_sc,
            tc.tile_pool(name="ps_p", bufs=1, space="PSUM") as ps_p,
            tc.tile_pool(name="ps_aux", bufs=1, space="PSUM") as ps_aux,
            tc.tile_pool(name="dram", bufs=1, space="DRAM") as dp,
        ):
            # ---- persistent SBUF tensors ----
            xT = pp.tile([128, 8 * S], bf, tag="xT")          # 64KB/part
            w1 = pp.tile([128, 8 * 128], bf, tag="w1")
            wv = pp.tile([128, 8 * HD], bf, tag="wv")
            At = pp.tile([128, S], bf, tag="At")
            Bt = pp.tile([128, S], bf, tag="Bt")
            mk = pp.tile([128, 128], bf, tag="mk")  # one causal triangle
            wo = pp.tile([128, 2 * D], bf, tag="wo")          # head-pair c rows 128c
            rot = pp.tile([128, S], bf, tag="rot")            # rows 0:64 q~, 64:128 k~
            rotB = pp.tile([128, S], bf, tag="rotB")          # swapped: k~ low, q~ high
            VS = HD + 1
            vaug = pp.tile([128, 32 * VS], bf, tag="vaug")
            outTs = []
            for q in range(NSB):
                oT = pp.tile([HD, SB], bf, tag=f"outT{q}")
                outTs.append(oT)
            idn = pp.tile([HD, HD], f32, tag="idn")
            permT = pp.tile([128, 128], bf, tag="permT")
            swapT = pp.tile([128, 128], bf, tag="swapT")
            ones65 = pp.tile([HD + 1, HD], f32, tag="ones65")
            wdbg = None
            if lvl < 4:
                # debug levels: tiny constant output write (timing-only levels;
                # content is irrelevant, keep it off the DMA-ring hot path)
                wdbg = pp.tile([128, 16], bf, tag="wdbg")

            def emit_body():
                # ---- input loads (all HWDGE: sync=SP ring for x, scalar=ACT
                # ring for weights/tables; all pre-cast to bf16 on host) ----
                # weights first (small, needed by the first projection)
                w1_3 = w1[:, :].rearrange("p (c m) -> p c m", c=8)
                w1d3 = w1_d[:, :].rearrange("(c p) m -> p c m", p=128)
                # w1 and the first x s-block stream in matching d-chunk order
                # so P(0)'s accumulation starts at the first-chunk mark
                nc.scalar.dma_start(w1_3[:, 0:2, :], w1d3[:, 0:2, :])
                nc.scalar.dma_start(w1_3[:, 2:8, :], w1d3[:, 2:8, :])
                xT3 = xT[:, :].rearrange("p (c s) -> p c s", c=8)
                xTd3 = xT_d[:, :].rearrange("(c p) s -> p c s", p=128)
                for c0 in range(0, 8, 2):
                    nc.sync.dma_start(xT3[:, c0:c0 + 2, 0: SB],
                                      xTd3[:, c0:c0 + 2, 0: SB])
                wv_3 = wv[:, :].rearrange("p (c m) -> p c m", c=8)
                wvd3 = wv_d[:, :].rearrange("(c p) m -> p c m", p=128)
                wo_3 = wo[:, :].rearrange("p (c n) -> p c n", c=2)
                wod3 = wo_d[:, :].rearrange("(c p) n -> p c n", p=128)
                if ldmode == 0:
                    # whole-table loads on the ACT ring
                    nc.scalar.dma_start(wv_3[:, :, :], wvd3[:, :, :])
                    nc.scalar.dma_start(At[:, :], ab_d[0, :, :])
                    nc.scalar.dma_start(Bt[:, :], ab_d[1, :, :])
                    nc.scalar.dma_start(mk[:, :], mk_d[:, :])
                else:
                    # latency-ordered ACT ring: only block-0 table slices up
                    # front; later At/Bt blocks ride inside the P(j) generators
                    nc.scalar.dma_start(At[:, 0:SB], ab_d[0, :, 0:SB])
                    nc.scalar.dma_start(Bt[:, 0:SB], ab_d[1, :, 0:SB])
                    nc.scalar.dma_start(wv_3[:, :, :], wvd3[:, :, :])
                    nc.scalar.dma_start(mk[:, :], mk_d[:, :])
                nc.scalar.dma_start(idn[:, :], idn_d[:, :])
                nc.scalar.dma_start(permT[:, :], permT_d[:, :])
                nc.scalar.dma_start(swapT[:, :], swapT_d[:, :])
                nc.vector.memset(ones65[:, :], 1.0)
                for c in range(32):
                    nc.vector.memset(vaug[:, VS * c + HD: VS * c + HD + 1], 1.0)
                # s-sliced x loads: one DMA brings ALL 8 D-chunks for one
                # s-block, so block-j projection starts after load j
                for j in range(1, NSB):
                    nc.sync.dma_start(xT3[:, :, SB * j: SB * (j + 1)],
                                      xTd3[:, :, SB * j: SB * (j + 1)])

                if lvl < 1:
                    nc.vector.memset(wdbg[:, :], 1.0)
                    nc.sync.dma_start(out_d[0, 0:128, 0:16], wdbg[:, :])
                if lvl >= 3:
                    bin_ = dp.tile([NCORES, HD, 512], bf, tag="bin")
                    bout = dp.tile([NCORES, HD, 512], bf, tag="bout")

                # ---- stages P+A interleaved per s-block ----
                # P(j): projections + RoPE + V-transpose for s-block j.
                # A(qb=j): full attention row-band for q-block j (needs rot/v of
                # s-blocks 0..j only, all available after P(j)).
                # P(j+1) is emitted as a GENERATOR whose steps are pulled a few
                # at a time inside A(j)'s group loop: A is ACT(exp)-bound with
                # PE slack, so P's matmuls hide under the exp stream instead of
                # serializing between attention bands.
                def emit_P_steps(j):
                    sc = ps_p.tile([128, 3 * SB], f32, tag="pp")
                    if ldmode != 0 and j >= 1:
                        nc.scalar.dma_start(At[:, SB * j: SB * (j + 1)],
                                            ab_d[0, :, SB * j: SB * (j + 1)])
                        nc.scalar.dma_start(Bt[:, SB * j: SB * (j + 1)],
                                            ab_d[1, :, SB * j: SB * (j + 1)])

                    def xs(d8, _j=j):
                        return xT[:, S * d8 + SB * _j: S * d8 + SB * (_j + 1)]

                    for d8 in range(8):
                        nc.tensor.matmul(sc[:, 0:512],
                                         w1[:, 128 * d8: 128 * (d8 + 1)],
                                         xs(d8), start=(d8 == 0), stop=(d8 == 7))
                        yield
                    for d8 in range(8):
                        nc.tensor.matmul(sc[0:HD, 1024:1536],
                                         wv[:, HD * d8: HD * (d8 + 1)],
                                         xs(d8), start=(d8 == 0), stop=(d8 == 7))
                        yield
                    # P2 = signed pair-swap of P1 -> ONE permutation matmul
                    p1s = wp.tile([128, SB], bf, tag="p1s")
                    nc.vector.tensor_copy(p1s[:, :], sc[:, 0:512])
                    nc.tensor.matmul(sc[:, 512:1024], permT[:, :], p1s[:, :],
                                     start=True, stop=True)
                    yield
                    u = wp.tile([128, SB], bf, tag="u")
                    w_ = wp.tile([128, SB], f32, tag="w_")
                    nc.vector.tensor_mul(u[:, :], p1s[:, :],
                                         At[:, SB * j: SB * (j + 1)])
                    nc.vector.tensor_mul(w_[:, :], sc[:, 512:1024],
                                         Bt[:, SB * j: SB * (j + 1)])
                    nc.vector.tensor_add(rot[:, SB * j: SB * (j + 1)],
                                         u[:, :], w_[:, :])
                    yield
                    # partition-swapped copy rotB (k~ low / q~ high) via one PE
                    # matmul + one DVE copy: no DMA on any ring, so the ACT
                    # queue stays pure exp during attention (head-of-line!)
                    nc.tensor.matmul(sc[:, 512:1024], swapT[:, :],
                                     rot[:, SB * j: SB * (j + 1)],
                                     start=True, stop=True)
                    yield
                    nc.vector.tensor_copy(rotB[:, SB * j: SB * (j + 1)],
                                          sc[:, 512:1024])
                    yield
                    vts = wp.tile([HD, SB], f32, tag="vts")
                    nc.vector.tensor_copy(vts[:, :], sc[0:HD, 1024:1536])
                    yield
                    # transposes reuse this j's pp bank 0 (P1 already consumed)
                    for t in range(4):
                        nc.tensor.transpose(sc[:, HD * t: HD * (t + 1)],
                                            vts[:, 128 * t: 128 * (t + 1)],
                                            idn[:, :])
                        yield
                    for t in range(4):
                        cch = 4 * j + t
                        nc.vector.tensor_copy(vaug[:, VS * cch: VS * cch + HD],
                                              sc[:, HD * t: HD * (t + 1)])

                def drain(gen):
                    if gen is not None:
                        for _ in gen:
                            pass

                if lvl >= 1:
                    drain(emit_P_steps(0))
                pending_tail = None
                for qb in range(NSB if lvl >= 2 else 0):
                    if qb >= 1 and pulls == 0:
                        drain(emit_P_steps(qb))  # serial P(j);A(j) ordering
                    pgen = (emit_P_steps(qb + 1)
                            if (qb + 1 < NSB and pulls > 0) else None)
                    # ---- A(qb) ----
                    po = ps_aux.tile([HD + 1, 512], f32, tag="po")
                    nk = 4 * (qb + 1)
                    for g0 in range(0, nk, GRP):
                        cnt = min(GRP, nk - g0)
                        sc = ps_sc.tile([128, GRP * SB], f32, tag="sc")
                        for r in range(cnt):
                            kb = g0 + r
                            # diagonal chunk di>=1: queries < 128*di are fully
                            # masked -> restrict scores/AV to cols >= 128*di
                            di = kb - (nk - 4)
                            lo = 128 * di if di >= 1 else 0
                            dst = sc[:, 512 * r + lo: 512 * (r + 1)]
                            if r % 2 == 0:
                                # row-tile T0: k~ (rotB low) x q~ (rot low)
                                nc.tensor.matmul(
                                    dst, rotB[0:HD, 128 * kb: 128 * (kb + 1)],
                                    rot[0:HD, SB * qb + lo: SB * (qb + 1)],
                                    start=True, stop=True)
                            else:
                                # row-tile T8: k~ (rot high) x q~ (rotB high)
                                nc.tensor.matmul(
                                    dst, rot[64:128, 128 * kb: 128 * (kb + 1)],
                                    rotB[64:128, SB * qb + lo: SB * (qb + 1)],
                                    start=True, stop=True)
                        pe = ep.tile([128, GRP * SB], bf, tag="pe")
                        nc.scalar.activation(pe[:, 0: 512 * cnt],
                                             sc[:, 0: 512 * cnt], EXP)
                        # pull a few P(qb+1) steps: their PE matmuls land
                        # between this group's scores and AV in the in-order
                        # PE queue, filling the wait for exp(group) to finish
                        if pgen is not None:
                            for _ in range(pulls):
                                if next(pgen, "end") == "end":
                                    pgen = None
                                    break
                        if pending_tail is not None:
                            # deferred qb-1 normalization: emitted after this
                            # group's scores/exp so the PE does not stall on
                            # the reciprocal, but BEFORE this qb's first AV
                            # (whose po WAR-dep must see these po readers)
                            pending_tail()
                            pending_tail = None
                        for r in range(cnt):
                            di = (g0 + r) - (nk - 4)
                            if di >= 0:
                                # only the 128-wide triangle strip needs the
                                # mask (cols < 128di skipped by scores/AV,
                                # cols >= 128(di+1) are all-valid)
                                o = 512 * r + 128 * di
                                nc.vector.tensor_mul(pe[:, o: o + 128],
                                                     pe[:, o: o + 128],
                                                     mk[:, :])
                        for r in range(cnt):
                            kb = g0 + r
                            di = kb - (nk - 4)
                            lo = 128 * di if di >= 1 else 0
                            nc.tensor.matmul(po[:, lo: 512],
                                             vaug[:, VS * kb: VS * kb + HD + 1],
                                             pe[:, 512 * r + lo: 512 * (r + 1)],
                                             start=(kb == 0), stop=(kb == nk - 1))
                    drain(pgen)

                    def emit_tail(_qb=qb, _po=po):
                        # normalize locally BEFORE the a2a: recip of the sums
                        # row (partition 64) into SBUF, matmul-broadcast
                        rcs = wp.tile([HD + 1, 512], f32, tag="rcs")
                        nc.vector.reciprocal(rcs[HD:HD + 1, :],
                                             _po[HD:HD + 1, :])
                        bc = ps_sc.tile([HD, 512], f32, tag="sc")
                        nc.tensor.matmul(bc[:, :], ones65[HD:HD + 1, :],
                                         rcs[HD:HD + 1, :],
                                         start=True, stop=True)
                        # walrus: a DVE op may read at most ONE PSUM operand
                        nc.any.tensor_copy(outTs[_qb][:, :], _po[0:HD, :])
                        nc.vector.tensor_mul(outTs[_qb][:, :],
                                             outTs[_qb][:, :], bc[:, :])
                        if lvl >= 3:
                            # eager bounce-out: slice qb of outT = a2a block
                            nc.sync.dma_start(bin_[_qb, :, :], outTs[_qb][:, :])

                    if qb + 1 < (NSB if lvl >= 2 else 0):
                        pending_tail = emit_tail
                    else:
                        emit_tail()
                if pending_tail is not None:
                    pending_tail()
                    pending_tail = None
                if lvl == 1:
                    for j in range(1, NSB):
                        drain(emit_P_steps(j))

                # ---- stage C: AllToAll (512-col output slices, all 8 cores) --
                if lvl in (1, 2):
                    nc.vector.memset(wdbg[:, :], 1.0)
                    nc.sync.dma_start(out_d[0, 0:128, 0:16], wdbg[:, :])
                if lvl >= 3:
                    if collective:
                        from concourse import mybir as _mb
                        nc.gpsimd.collective_compute(
                            "AllToAll", _mb.AluOpType.bypass,
                            replica_groups=[list(range(NCORES))],
                            ins=[bin_.opt()], outs=[bout.opt()],
                        )
                    else:
                        # single-core timeline-sim stand-in: local DRAM move
                        nc.sync.dma_start(bout[:, :, :], bin_[:, :, :])
                    if lvl == 3:
                        nc.vector.memset(wdbg[:, :], 1.0)
                        nc.sync.dma_start(out_d[0, 0:128, 0:16], wdbg[:, :])

                # ---- stage D: wo matmul + output (already normalized) ----
                if lvl >= 4:
                    # wo load deferred to here: first needed by the D matmuls,
                    # keeps it off the early HBM window and the ACT ring short
                    nc.scalar.dma_start(wo_3[:, :, :], wod3[:, :, :])
                    # head pairs stacked on partition halves -> K=128 wo matmul
                    g2 = pp.tile([128, 4 * 512], bf, tag="g2")
                    g2lo = g2[0:HD, :].rearrange("p (m s) -> p m s", m=4)
                    g2hi = g2[HD:128, :].rearrange("p (m s) -> p m s", m=4)
                    bt3 = bout[:, :, :].rearrange("(m e) p s -> e p m s", e=2)
                    nc.sync.dma_start(g2lo[:, :, :], bt3[0, :, :, :])
                    nc.scalar.dma_start(g2hi[:, :, :], bt3[1, :, :, :])
                    for b in range(B):
                        for t in range(4):
                            # bf16 staging halves both the output DMA and the
                            # host fetch; rel-err cost ~4e-3, well inside gate
                            ys = wp.tile([128, 1024], bf, tag="ys")
                            for nh in range(2):
                                # stage-A sc slots are free here; reuse for 2x
                                # double-buffered wo-psum
                                yp = ps_sc.tile([128, 512], f32, tag="sc")
                                for pr in range(2):
                                    m = 2 * b + pr
                                    nc.tensor.matmul(
                                        yp[:, :],
                                        g2[:, 512 * m + 128 * t:
                                           512 * m + 128 * (t + 1)],
                                        wo[:, D * pr + 512 * nh:
                                           D * pr + 512 * (nh + 1)],
                                        start=(pr == 0), stop=(pr == 1))
                                nc.vector.tensor_copy(ys[:, 512 * nh:
                                                         512 * (nh + 1)],
                                                      yp[:, :])
                                # per-half write: drains the last tile sooner
                                eng = (nc.sync, nc.scalar)[(t + nh) % 2]
                                eng.dma_start(
                                    out_d[b, 128 * t: 128 * (t + 1),
                                          512 * nh: 512 * (nh + 1)],
                                    ys[:, 512 * nh: 512 * (nh + 1)])

            for _it in range(iters):
                emit_body()

    nc.compile()
    return nc


def _get_nc():
    if "nc" not in _CACHE:
        _CACHE["nc"] = _build_nc()
    return _CACHE["nc"]


def _prep_in_maps(x, wq, wk, wv, wo, freq_cos, freq_sin):
    x = np.asarray(x, np.float32)
    wq = np.asarray(wq, np.float32)
    wk = np.asarray(wk, np.float32)
    wv = np.asarray(wv, np.float32)
    wo = np.asarray(wo, np.float32)
    cos = np.asarray(freq_cos, np.float32)
    sin = np.asarray(freq_sin, np.float32)

    # group-sum wq per kv head (einsum sums over group axis); fold both /8 scales
    wqr = wq.reshape(D, QH, HD)
    wq_eff = np.stack([wqr[:, h::KVH].sum(axis=1) for h in range(KVH)], axis=1) / 64.0
    wkr = wk.reshape(D, KVH, HD)
    W1 = np.empty((KVH, D, 128), np.float32)
    for h in range(KVH):
        q1, q2 = wq_eff[:, h, 0::2], wq_eff[:, h, 1::2]
        k1, k2 = wkr[:, h, 0::2], wkr[:, h, 1::2]
        W1[h] = np.concatenate([q1, q2, k1, k2], axis=1)
    Wv = np.ascontiguousarray(wv.reshape(D, KVH, HD).transpose(1, 0, 2))

    A = np.tile(cos.T, (4, 1)).astype(ml_dtypes.bfloat16)   # [128, S]
    Bm = np.tile(sin.T, (4, 1)).astype(ml_dtypes.bfloat16)
    ab = np.ascontiguousarray(np.stack([A, Bm]))

    qi = np.arange(128)[None, :]
    ki = np.arange(128)[:, None]
    masks = np.ascontiguousarray((qi >= ki).astype(ml_dtypes.bfloat16))

    xTb = [np.ascontiguousarray(x[b].T).astype(ml_dtypes.bfloat16) for b in range(B)]
    wo_bf = np.ascontiguousarray(wo.astype(ml_dtypes.bfloat16))

    in_maps = []
    for c in range(NCORES):
        b, h = c // KVH, c % KVH
        in_maps.append({
            "xT": xTb[b],
            "w1": np.ascontiguousarray(W1[h].astype(ml_dtypes.bfloat16)),
            "wv": np.ascontiguousarray(Wv[h].astype(ml_dtypes.bfloat16)),
            "ab": ab,
            "masks": masks,
            "wo": wo_bf,
        })
    return in_maps


def _assemble(results):
    full = np.empty((B, S, D), np.float32)
    for c in range(NCORES):
        y = results[c]["out"]  # [B, 512, D] bf16
        for b in range(B):
            full[b, SB * c: SB * (c + 1), :] = y[b].astype(np.float32)
    return full


def _ensure_axon_hooks_stub():
    # slim axon builds lack antenv.axon_hooks; degrade trace=True gracefully
    try:
        import antenv.axon_hooks  # noqa: F401
    except Exception:
        import types
        m = types.ModuleType("antenv.axon_hooks")
        m.get_axon_ntff_profile_hook = lambda: None
        sys.modules["antenv.axon_hooks"] = m


def run(in_maps, trace=False):
    from concourse.bass_utils import run_bass_kernel_spmd
    _ensure_axon_hooks_stub()
    nc = _get_nc()
    res = run_bass_kernel_spmd(nc, in_maps, core_ids=list(range(NCORES)),
                               trace=trace)
    return res


def kernel(**inputs):
    in_maps = _prep_in_maps(**inputs)
    res = run(in_maps, trace=False)
    return _assemble(res.results)


if __name__ == "__main__":
    # smoke: build only
    _get_nc()
    print("built ok")
